# revision 8
# baseline (speedup 1.0000x reference)
"""Multi-head attention block (QKV projections + softmax attention + output
projection) for TRN2, distributed over 8 NeuronCores.

Sharding: core c handles batch b = c // 2 and head group g = c % 2 (8 of
the 16 heads).  Data parallel over batch, tensor parallel over heads.
Each core computes a partial output (its head group's contribution to the
final projection); the host sums the two partials per batch and adds the
output bias.

Layout strategy (all matmuls contract over the SBUF partition dim):
  - Host sends q/k/v pre-transposed (qT = q[b].T, [D, n]) so projections
    need no on-device transposes.
  - Q'T, K'T are produced head-major-transposed ([j, n], j = head*64+d)
    directly by the projection matmuls; V' is produced in natural [k, j]
    layout with a constant-1 column appended per head, so the P@V matmul
    also emits the softmax denominator for free.
  - Scores are computed transposed (S^T[k, q]) with head pairs row-tiled
    in the PE array (even head on partitions 0-63, odd on 64-127); exp
    runs on the scalar engine PSUM->SBUF; normalization happens on the
    small O^T tiles (64 x 512) after the P@V matmul.
  - All matmuls run in float32r (fp32 data, ~2^-13 effective mantissa in
    the PE, 1 column/cycle at N>=256 -- 4x faster than plain fp32).

Scheduling: inputs stream in 512 KB chunks on one DMA queue in first-use
order; projections run contraction-chunk-outer across 8 concurrently
accumulating PSUM banks so compute tracks DMA chunk arrivals; the V
projection and the first score blocks interleave so the PE never waits
on the vT DMA tail; in the attention drain, each cycle fuses the
next score block with the previous block's P@V at kc-pair granularity,
so the PE always has ready P@V work while score matmuls wait on exp
slot releases.

The attention scale (all_head_dim**-0.5) and its effect on bq are folded
into the Q weights on the host.  The attention mask is applied as an
additive score bias only when it is not all-ones; biases are applied (as
rank-1 matmul accumulands) only when nonzero.
"""

from collections import deque
from contextlib import ExitStack

import numpy as np

import concourse.bass as bass
import concourse.tile as tile
from concourse import bacc, mybir
from concourse.bass_utils import run_bass_kernel_spmd

# Problem shape (hardcoded per the harness contract).
NUM_HEADS = 16
B = 4
N = 1024          # sequence length (n_q == n_k)
D = 1024          # model dim
OUT = 1024        # output dim
HD = 64           # head dim
NH = 8            # heads per core (16 heads / 2 groups)
J = NH * HD       # per-core all-head dim = 512

P = 128           # SBUF partitions
ND = D // P       # 8 contraction chunks over D
NKT = N // P      # 8 key-token chunks
NQM = N // P      # 8 query-row chunks
NJ = J // P       # 4 chunks over j
QW = 512          # matmul moving width / PSUM bank width (fp32)
NQC = N // QW     # 2 query-column halves

F32 = mybir.dt.float32
F32R = mybir.dt.float32r


def _build(use_mask: bool, use_bias: bool, reps: int = 1):
    nc = bacc.Bacc(None, target_bir_lowering=False)

    def din(nm, shape, dt=F32R):
        return nc.declare_dram_parameter(nm, shape, dt, isOutput=False)

    qt_d, kt_d, vt_d = din("qt", [D, N]), din("kt", [D, N]), din("vt", [D, N])
    wq_d, wk_d, wv_d = din("wq", [D, J]), din("wk", [D, J]), din("wv", [D, J])
    wo_d = din("wo", [J, OUT])
    if use_bias:
        bqs_d, bks_d, bvs_d = (
            din("bqs", [1, J]), din("bks", [1, J]), din("bvs", [1, J])
        )
    mb_d = din("mb", [N, N], F32) if use_mask else None
    out_d = nc.declare_dram_parameter("out", [N, OUT], F32, isOutput=True)

    AF = mybir.ActivationFunctionType
    big_bufs = 18 if use_mask else 22

    with tile.TileContext(nc) as tc:
        with ExitStack() as ctx:
            # 512 KB streaming slots: input chunks early, P^T tiles later.
            big = ctx.enter_context(tc.tile_pool(name="big", bufs=big_bufs))
            pacts = ctx.enter_context(tc.tile_pool(name="acts", bufs=1))
            pwo = ctx.enter_context(tc.tile_pool(name="pwo", bufs=1))
            pout = ctx.enter_context(tc.tile_pool(name="outc", bufs=2))
            psml = ctx.enter_context(tc.tile_pool(name="small", bufs=1))
            # All PSUM tiles are 2-bank [P, 2, QW]; 4 slots = all 8 banks.
            psum = ctx.enter_context(tc.tile_pool(name="ps", bufs=3, space="PSUM"))
            if use_mask:
                pmask = ctx.enter_context(tc.tile_pool(name="pmask", bufs=1))

            # Small persistent tiles.
            onesf = psml.tile([1, QW], F32, name="onesf", tag="onesf")
            nc.vector.memset(onesf[:], 1.0)
            if use_bias:
                ones_row = psml.tile([1, QW], F32R, name="ones_row", tag="ones_row")
                nc.vector.tensor_copy(ones_row[:], onesf[:])
                bqs_t = psml.tile([1, J], F32R, name="bqs_t", tag="bqs")
                bks_t = psml.tile([1, J], F32R, name="bks_t", tag="bks")
                bvs_t = psml.tile([1, J], F32R, name="bvs_t", tag="bvs")
                nc.sync.dma_start(bqs_t[:], bqs_d[:])
                nc.sync.dma_start(bks_t[:], bks_d[:])
                nc.sync.dma_start(bvs_t[:], bvs_d[:])
            ones_hd = psml.tile([1, HD], F32R, name="ones_hd", tag="ones_hd")
            nc.vector.tensor_copy(ones_hd[:], onesf[0:1, 0:HD])
            vof = psml.tile([P, NKT, NH, 1], F32, name="vof", tag="vof")
            nc.vector.memset(vof[:], 1.0)

            # Warm the ACT exp table while DMAs run.
            warm = psml.tile([1, QW], F32, name="warm", tag="rc", bufs=2)
            nc.scalar.activation(warm[:], onesf[:], AF.Exp)

            if use_mask:
                mb_t = pmask.tile([P, NKT, N], F32, name="mb_t", tag="mask")
                nc.sync.dma_start(
                    mb_t[:], mb_d[:].rearrange("(a p) n -> p a n", p=P)
                )

            def _emit_rep():
                # --- streamed input chunks (512 KB each), first-use order.
                # w chunks: [P, 2, J] covering 2 D-chunks; x chunks: [P, N]
                # covering 1 D-chunk.
                def load_wx(wd, xd, nm):
                    w, x = [], []
                    for i in range(4):
                        t = big.tile([P, 2, J], F32R, name=f"w{nm}{i}", tag="big")
                        w.append(t)
                    for i in range(ND):
                        t = big.tile([P, N], F32R, name=f"x{nm}{i}", tag="big")
                        x.append(t)
                    order = [(w, 0), (x, 0), (x, 1), (w, 1), (x, 2), (x, 3),
                             (w, 2), (x, 4), (x, 5), (w, 3), (x, 6), (x, 7)]
                    for lst, i in order:
                        if lst is w:
                            nc.sync.dma_start(
                                w[i][:],
                                wd[i * (2 * P):(i + 1) * (2 * P), :].rearrange(
                                    "(a p) j -> p a j", p=P
                                ),
                            )
                        else:
                            nc.sync.dma_start(x[i][:], xd[i * P:(i + 1) * P, :])
                    return w, x

                def wslice(w, dc, cols):
                    return w[dc // 2][:, dc % 2, cols]

                qpt = pacts.tile([P, NJ, N], F32R, name="qpt", tag="qpt")
                kpt = pacts.tile([P, NJ, N], F32R, name="kpt", tag="kpt")
                vext = pacts.tile([P, NKT, NH, HD + 1], F32R, name="vext",
                                  tag="vext")
                ot = pacts.tile([P, NJ, N], F32R, name="ot", tag="ot")

                # --- Q/K projections, contraction-chunk outer over 8
                # concurrently accumulating PSUM banks (4 two-bank tiles).
                def qk_proj(nm, wd, xd, bias_t, dst):
                    w, x = load_wx(wd, xd, nm)
                    for cs in (range(3), range(3, NJ)):
                        groups = {
                            c: psum.tile([P, NQC, QW], F32, name="psp",
                                         tag="ps")
                            for c in cs
                        }
                        if use_bias:
                            for c in cs:
                                for qc in range(NQC):
                                    nc.tensor.matmul(
                                        groups[c][:, qc, :],
                                        bias_t[0:1, c * P:(c + 1) * P],
                                        ones_row[:], start=True, stop=False,
                                        skip_group_check=True,
                                    )
                        for dc in range(ND):
                            for c in cs:
                                for qc in range(NQC):
                                    nc.tensor.matmul(
                                        groups[c][:, qc, :],
                                        wslice(w, dc, slice(c * P, (c + 1) * P)),
                                        x[dc][:, qc * QW:(qc + 1) * QW],
                                        start=(dc == 0 and not use_bias),
                                        stop=(dc == ND - 1),
                                        skip_group_check=True,
                                    )
                        for c in cs:
                            nc.vector.tensor_copy(
                                dst[:, c, :],
                                groups[c][:].rearrange("p a q -> p (a q)"),
                            )

                qk_proj("q", wq_d, qt_d, bqs_t if use_bias else None, qpt)
                qk_proj("k", wk_d, kt_d, bks_t if use_bias else None, kpt)

                # --- Attention blocks.  Head pairs are row-tiled in the PE
                # (even head partitions 0-63, odd 64-127); exps run over
                # two score banks at once ([128, 1024]).
                def s_block(pr, qc, pts):
                    for h in (2 * pr, 2 * pr + 1):
                        pts[(h, qc)] = [
                            big.tile([P, 2, QW], F32R, name="pt", tag="big")
                            for _ in range(NKT // 2)
                        ]
                    for kcp in range(NKT // 2):
                        pss = {}
                        for h in (2 * pr, 2 * pr + 1):
                            pss[h] = psum.tile([P, 2, QW], F32, name="pss",
                                               tag="ps")
                        for i in range(2):
                            kc = 2 * kcp + i
                            for h in (2 * pr, 2 * pr + 1):
                                off = HD * (h & 1)
                                nc.tensor.matmul(
                                    pss[h][:, i, :],
                                    kpt[off:off + HD, pr, kc * P:(kc + 1) * P],
                                    qpt[off:off + HD, pr, qc * QW:(qc + 1) * QW],
                                    start=True, stop=True,
                                    skip_group_check=True,
                                )
                        for h in (2 * pr, 2 * pr + 1):
                            if use_mask:
                                nc.vector.tensor_add(
                                    pss[h][:],
                                    pss[h][:],
                                    mb_t[:, 2 * kcp:2 * kcp + 2,
                                         qc * QW:(qc + 1) * QW],
                                )
                            nc.scalar.activation(
                                pts[(h, qc)][kcp][:], pss[h][:], AF.Exp,
                            )

                def pv_block(h, qc, pt):
                    po = psum.tile([HD + 1, QW], F32, name="po", tag="po",
                                   bufs=2)
                    for kc in range(NKT):
                        nc.tensor.matmul(
                            po[:], vext[:, kc, h, :],
                            pt[kc // 2][:, kc % 2, :],
                            start=(kc == 0), stop=(kc == NKT - 1),
                        )
                    rc = psml.tile([1, QW], F32, name="rc", tag="rc", bufs=2)
                    nc.vector.reciprocal(rc[:], po[HD:HD + 1, :])
                    rb = psml.tile([HD, QW], F32, name="rb", tag="rb", bufs=2)
                    nc.gpsimd.partition_broadcast(rb[:], rc[:])
                    off = HD * (h & 1)
                    nc.vector.tensor_mul(
                        ot[off:off + HD, h // 2, qc * QW:(qc + 1) * QW],
                        po[0:HD, :], rb[:],
                    )

                def fused_block(s_task, pv_task, pts):
                    """One drain cycle: score block for s_task with the
                    P@V pair for pv_task interleaved at kc-pair steps, so
                    the PE has ready P@V work whenever score matmuls wait
                    on an exp slot release."""
                    pr, qc = s_task
                    prv, qcv = pv_task
                    pta = pts.pop((2 * prv, qcv))
                    ptb = pts.pop((2 * prv + 1, qcv))
                    poa = psum.tile([HD + 1, QW], F32, name="po", tag="po",
                                    bufs=2)
                    pob = psum.tile([HD + 1, QW], F32, name="po", tag="po",
                                    bufs=2)
                    for h in (2 * pr, 2 * pr + 1):
                        pts[(h, qc)] = [
                            big.tile([P, 2, QW], F32R, name="pt", tag="big")
                            for _ in range(NKT // 2)
                        ]
                    for kcp in range(NKT // 2):
                        pss = {}
                        for h in (2 * pr, 2 * pr + 1):
                            pss[h] = psum.tile([P, 2, QW], F32, name="pss",
                                               tag="ps")
                        for i in range(2):
                            kc = 2 * kcp + i
                            for h in (2 * pr, 2 * pr + 1):
                                off = HD * (h & 1)
                                nc.tensor.matmul(
                                    pss[h][:, i, :],
                                    kpt[off:off + HD, pr, kc * P:(kc + 1) * P],
                                    qpt[off:off + HD, pr, qc * QW:(qc + 1) * QW],
                                    start=True, stop=True,
                                    skip_group_check=True,
                                )
                        for h in (2 * pr, 2 * pr + 1):
                            if use_mask:
                                nc.vector.tensor_add(
                                    pss[h][:],
                                    pss[h][:],
                                    mb_t[:, 2 * kcp:2 * kcp + 2,
                                         qc * QW:(qc + 1) * QW],
                                )
                            nc.scalar.activation(
                                pts[(h, qc)][kcp][:], pss[h][:], AF.Exp,
                            )
                        for po_, pt_, hv in ((poa, pta, 2 * prv),
                                             (pob, ptb, 2 * prv + 1)):
                            for i in range(2):
                                kc = 2 * kcp + i
                                nc.tensor.matmul(
                                    po_[:], vext[:, kc, hv, :],
                                    pt_[kc // 2][:, kc % 2, :],
                                    start=(kc == 0), stop=(kc == NKT - 1),
                                )
                    for po_, hv in ((poa, 2 * prv), (pob, 2 * prv + 1)):
                        rc = psml.tile([1, QW], F32, name="rc", tag="rc",
                                       bufs=2)
                        nc.vector.reciprocal(rc[:], po_[HD:HD + 1, :])
                        rb = psml.tile([HD, QW], F32, name="rb", tag="rb",
                                       bufs=2)
                        nc.gpsimd.partition_broadcast(rb[:], rc[:])
                        off = HD * (hv & 1)
                        nc.vector.tensor_mul(
                            ot[off:off + HD, hv // 2, qcv * QW:(qcv + 1) * QW],
                            po_[0:HD, :], rb[:],
                        )

                pts = {}
                squeue = deque(
                    (pr, qc) for qc in range(NQC) for pr in range(NH // 2)
                )
                pvqueue = deque()

                def emit_s_one():
                    if squeue:
                        pr, qc = squeue.popleft()
                        s_block(pr, qc, pts)
                        pvqueue.append((pr, qc))

                def emit_pv_one():
                    pr, qc = pvqueue.popleft()
                    pv_block(2 * pr, qc, pts.pop((2 * pr, qc)))
                    pv_block(2 * pr + 1, qc, pts.pop((2 * pr + 1, qc)))

                # --- V projection (into [k, j] + ones column) in two
                # passes over 3 two-bank PSUM tiles, one score block
                # interleaved so ACT gets exp work during the vT DMA.
                vw, vx = load_wx(wv_d, vt_d, "v")
                nc.vector.tensor_copy(vext[:, :, :, HD:HD + 1], vof[:])
                for kcs in (range(0, 6), range(6, NKT)):
                    vgroups = {
                        kc: psum.tile([P, 2, QW], F32, name="psv", tag="ps")
                        for kc in kcs[::2]
                    }
                    if use_bias:
                        for kc in kcs:
                            nc.tensor.matmul(
                                vgroups[kc - kc % 2][:, kc % 2, :],
                                ones_row[0:1, 0:P], bvs_t[:],
                                start=True, stop=False,
                                skip_group_check=True,
                            )
                    for dc in range(ND):
                        for kc in kcs:
                            nc.tensor.matmul(
                                vgroups[kc - kc % 2][:, kc % 2, :],
                                vx[dc][:, kc * P:(kc + 1) * P],
                                wslice(vw, dc, slice(0, J)),
                                start=(dc == 0 and not use_bias),
                                stop=(dc == ND - 1),
                                skip_group_check=True,
                            )
                        if kcs.start == 0 and dc == 3:
                            emit_s_one()
                    for kc in kcs[::2]:
                        nc.vector.tensor_copy(
                            vext[:, kc:kc + 2, :, 0:HD],
                            vgroups[kc][:].rearrange(
                                "p a (h d) -> p a h d", h=NH
                            ),
                        )

                wo_t = pwo.tile([P, NJ, OUT], F32R, name="wo_t", tag="wo")
                nc.sync.dma_start(
                    wo_t[:], wo_d[:].rearrange("(a p) n -> p a n", p=P)
                )

                # --- Output projection (partial over this core's heads),
                # emitted per query half as soon as that half's O^T is done.
                def emit_final_qm(qm):
                    ps = psum.tile([P, NQC, QW], F32, name="psf", tag="ps")
                    for oc in range(NQC):
                        for jc in range(NJ):
                            nc.tensor.matmul(
                                ps[:, oc, :],
                                ot[:, jc, qm * P:(qm + 1) * P],
                                wo_t[:, jc, oc * QW:(oc + 1) * QW],
                                start=(jc == 0), stop=(jc == NJ - 1),
                                skip_group_check=True,
                            )
                    oc_t = pout.tile([P, OUT], F32, name="oct", tag="outc")
                    nc.vector.tensor_copy(
                        oc_t[:], ps[:].rearrange("p a q -> p (a q)")
                    )
                    nc.sync.dma_start(out_d[qm * P:(qm + 1) * P, :], oc_t[:])

                def emit_final(qhalf):
                    for qm in range(4 * qhalf, 4 * qhalf + 4):
                        emit_final_qm(qm)

                # --- drain remaining score blocks and P@V, software
                # pipelined (PV for block i emitted after block i+1); the
                # qc=0 half of the output projection interleaves with the
                # qc=1 score blocks.
                done_pv = 0
                finq = deque()
                prev = pvqueue.popleft()  # block emitted during V-proj
                while squeue:
                    cur = squeue.popleft()
                    fused_block(cur, prev, pts)
                    done_pv += 1
                    if done_pv == NH // 2:
                        finq.extend(range(4))  # qc0 output rows ready
                    if finq:
                        emit_final_qm(finq.popleft())
                    prev = cur
                while finq:
                    emit_final_qm(finq.popleft())
                pv_block(2 * prev[0], prev[1], pts.pop((2 * prev[0], prev[1])))
                pv_block(2 * prev[0] + 1, prev[1],
                         pts.pop((2 * prev[0] + 1, prev[1])))
                emit_final(1)

            if reps == 1:
                _emit_rep()
            else:
                with tc.For_i(0, reps, 1):
                    _emit_rep()

    nc.compile()
    return nc


_NC_CACHE = {}


def _get_nc(use_mask: bool, use_bias: bool = False, reps: int = 1):
    key = (use_mask, use_bias, reps)
    if key not in _NC_CACHE:
        _NC_CACHE[key] = _build(use_mask, use_bias, reps)
    return _NC_CACHE[key]


def _group_weights(Wq, bq, Wk, bk, Wv, bv, Wo, g):
    """Per-head-group weight slices in per-core layout j = head*64 + d.

    The module splits heads as reshape(b, n, head_dim, NUM_HEADS), so
    global column d*NUM_HEADS + h belongs to (head h, dim d).
    """
    scale = float(NUM_HEADS * HD) ** -0.5
    cols = np.array(
        [d * NUM_HEADS + (NH * g + hl) for hl in range(NH) for d in range(HD)]
    )
    f = np.float32
    return {
        "wq": np.ascontiguousarray(Wq[:, cols] * scale, dtype=f),
        "bqs": np.ascontiguousarray((bq[cols] * scale)[None, :], dtype=f),
        "wk": np.ascontiguousarray(Wk[:, cols], dtype=f),
        "bks": np.ascontiguousarray(bk[cols][None, :], dtype=f),
        "wv": np.ascontiguousarray(Wv[:, cols], dtype=f),
        "bvs": np.ascontiguousarray(bv[cols][None, :], dtype=f),
        "wo": np.ascontiguousarray(Wo[cols, :], dtype=f),
    }


def make_in_maps(q, k, v, attn_mask, Wq, bq, Wk, bk, Wv, bv, Wo, bo):
    """Shard the full inputs into 8 per-core input maps."""
    use_mask = not bool(np.all(np.asarray(attn_mask) == 1.0))
    use_bias = bool(
        np.any(np.asarray(bq)) or np.any(np.asarray(bk)) or np.any(np.asarray(bv))
    )
    gw = [_group_weights(Wq, bq, Wk, bk, Wv, bv, Wo, g) for g in range(2)]
    f = np.float32
    xt = [
        {
            "qt": np.ascontiguousarray(np.asarray(q[b]).T, dtype=f),
            "kt": np.ascontiguousarray(np.asarray(k[b]).T, dtype=f),
            "vt": np.ascontiguousarray(np.asarray(v[b]).T, dtype=f),
        }
        for b in range(B)
    ]
    mb = None
    if use_mask:
        mb = np.ascontiguousarray(
            (-100000000.0 * (1.0 - np.asarray(attn_mask))).T, dtype=f
        )
    in_maps = []
    for c in range(8):
        b, g = divmod(c, 2)
        m = dict(xt[b])
        m.update(gw[g])
        if not use_bias:
            for nm in ("bqs", "bks", "bvs"):
                m.pop(nm, None)
        if use_mask:
            m["mb"] = mb
        in_maps.append(m)
    return in_maps, use_mask, use_bias


def kernel(q, k, v, attn_mask, Wq, bq, Wk, bk, Wv, bv, Wo, bo):
    in_maps, use_mask, use_bias = make_in_maps(
        q, k, v, attn_mask, Wq, bq, Wk, bk, Wv, bv, Wo, bo
    )
    nc = _get_nc(use_mask, use_bias)
    res = run_bass_kernel_spmd(nc, in_maps, list(range(8)))
    out = np.empty((B, N, OUT), np.float32)
    bo = np.asarray(bo, np.float32)
    for b in range(B):
        out[b] = res.results[2 * b]["out"] + res.results[2 * b + 1]["out"] + bo
    return out



# revision 9
# speedup vs baseline: 1.2754x; 1.2754x over previous
"""Multi-head attention block (QKV projections + softmax attention + output
projection) for TRN2, distributed over 8 NeuronCores.

Sharding: core c handles batch b = c // 2 and head group g = c % 2 (8 of
the 16 heads).  Data parallel over batch, tensor parallel over heads.
Each core computes a partial output (its head group's contribution to the
final projection); the host sums the two partials per batch and adds the
output bias.

All inputs are cast to bf16 on the host (rel err ~4e-3 vs the fp32
reference, well under tolerance); matmuls run bf16 x bf16 -> fp32 PSUM.
This halves DMA bytes and SBUF traffic vs fp32/f32r.

Layout strategy (all matmuls contract over the SBUF partition dim):
  - Host sends q/k/v pre-transposed (qT = q[b].T, [D, n]) so projections
    need no on-device transposes.
  - Q'T, K'T are produced head-major-transposed ([j, n], j = head*64+d)
    directly by the projection matmuls; V' is produced in natural [k, j]
    layout with a constant-1 column appended per head, so the P@V matmul
    also emits the softmax denominator for free.
  - Scores are computed transposed (S^T[k, q]) with head pairs row-tiled
    in the PE array (even head on partitions 0-63, odd on 64-127); exp
    runs on the scalar engine PSUM->SBUF (bf16 out); normalization
    happens on the small O^T tiles (64 x 512) after the P@V matmul.

Scheduling: inputs stream in 512 KB bf16 chunks on one DMA queue in
first-use order; projections run contraction-chunk-outer across 8
concurrently accumulating PSUM banks so compute tracks DMA chunk
arrivals; PSUM evacuation copies alternate between the scalar and vector
engines so neither serializes a projection boundary; the V projection
and the first score block interleave so the PE never waits on the vT DMA
tail; in the attention drain, each cycle fuses the next score block with
the previous block's P@V at kc-pair granularity, so the PE always has
ready P@V work while score matmuls wait on exp slot releases.

The attention scale (all_head_dim**-0.5) and its effect on bq are folded
into the Q weights on the host.  The attention mask is applied as an
additive score bias only when it is not all-ones; biases are applied (as
rank-1 matmul accumulands) only when nonzero.
"""

from collections import deque
from contextlib import ExitStack

import numpy as np
import ml_dtypes

import concourse.bass as bass
import concourse.tile as tile
from concourse import bacc, mybir
from concourse.bass_utils import run_bass_kernel_spmd

# Problem shape (hardcoded per the harness contract).
NUM_HEADS = 16
B = 4
N = 1024          # sequence length (n_q == n_k)
D = 1024          # model dim
OUT = 1024        # output dim
HD = 64           # head dim
NH = 8            # heads per core (16 heads / 2 groups)
J = NH * HD       # per-core all-head dim = 512
P = 128           # SBUF partitions
ND = D // P       # 8 contraction chunks over D
NKT = N // P      # 8 key-token chunks
NJ = J // P       # 4 chunks over j
QW = 512          # matmul moving width / PSUM bank width (fp32)
NQC = N // QW     # 2 query-column halves

F32 = mybir.dt.float32
BF16 = mybir.dt.bfloat16
NPBF = ml_dtypes.bfloat16


def _build(use_mask: bool, use_bias: bool, reps: int = 1):
    nc = bacc.Bacc(None, target_bir_lowering=False)

    def din(nm, shape, dt=BF16):
        return nc.declare_dram_parameter(nm, shape, dt, isOutput=False)

    qt_d, kt_d, vt_d = din("qt", [D, N]), din("kt", [D, N]), din("vt", [D, N])
    wq_d, wk_d, wv_d = din("wq", [D, J]), din("wk", [D, J]), din("wv", [D, J])
    wo_d = din("wo", [J, OUT])
    if use_bias:
        bqs_d, bks_d, bvs_d = (
            din("bqs", [1, J]), din("bks", [1, J]), din("bvs", [1, J])
        )
    mb_d = din("mb", [N, N], F32) if use_mask else None
    out_d = nc.declare_dram_parameter("out", [N, OUT], F32, isOutput=True)

    AF = mybir.ActivationFunctionType
    big_bufs = 12 if use_mask else 14

    with tile.TileContext(nc) as tc:
        with ExitStack() as ctx:
            # 512 KB streaming slots for input chunks.
            big = ctx.enter_context(tc.tile_pool(name="big", bufs=big_bufs))
            # 256 KB slots for exp'd P^T tiles (bf16).
            ppt = ctx.enter_context(tc.tile_pool(name="ppt", bufs=18))
            pacts = ctx.enter_context(tc.tile_pool(name="acts", bufs=2))
            pwo = ctx.enter_context(tc.tile_pool(name="pwo", bufs=1))
            pout = ctx.enter_context(tc.tile_pool(name="outc", bufs=2))
            psml = ctx.enter_context(tc.tile_pool(name="small", bufs=1))
            # All PSUM "ps" tiles are 2-bank [P, 2, QW]; po are 1-bank.
            psum = ctx.enter_context(tc.tile_pool(name="ps", bufs=3, space="PSUM"))
            if use_mask:
                pmask = ctx.enter_context(tc.tile_pool(name="pmask", bufs=1))

            # Small persistent tiles.
            onesf = psml.tile([1, QW], F32, name="onesf", tag="onesf")
            nc.vector.memset(onesf[:], 1.0)
            if use_bias:
                ones_row = psml.tile([1, QW], BF16, name="ones_row", tag="ones_row")
                nc.vector.tensor_copy(ones_row[:], onesf[:])
                bqs_t = psml.tile([1, J], BF16, name="bqs_t", tag="bqs")
                bks_t = psml.tile([1, J], BF16, name="bks_t", tag="bks")
                bvs_t = psml.tile([1, J], BF16, name="bvs_t", tag="bvs")
                nc.sync.dma_start(bqs_t[:], bqs_d[:])
                nc.sync.dma_start(bks_t[:], bks_d[:])
                nc.sync.dma_start(bvs_t[:], bvs_d[:])

            # Warm the ACT exp table while DMAs run.
            warm = psml.tile([1, QW], F32, name="warm", tag="rc", bufs=2)
            nc.scalar.activation(warm[:], onesf[:], AF.Exp)

            if use_mask:
                mb_t = pmask.tile([P, NKT, N], F32, name="mb_t", tag="mask")
                nc.sync.dma_start(
                    mb_t[:], mb_d[:].rearrange("(a p) n -> p a n", p=P)
                )

            def _emit_rep():
                # --- streamed input chunks (bf16), first-use order, with
                # small leading chunks so the first matmuls start early.
                # Bounds are (lo, hi) D-chunk ranges per DMA.
                W_BOUNDS = [(0, 1), (1, 4), (4, 8)]
                X_BOUNDS = [(0, 1), (1, 2), (2, 4), (4, 6), (6, 8)]

                def load_wx(wd, xd, nm):
                    w = [
                        big.tile([P, hi - lo, J], BF16, name=f"w{nm}{i}",
                                 tag="big")
                        for i, (lo, hi) in enumerate(W_BOUNDS)
                    ]
                    x = [
                        big.tile([P, hi - lo, N], BF16, name=f"x{nm}{i}",
                                 tag="big")
                        for i, (lo, hi) in enumerate(X_BOUNDS)
                    ]
                    order = [(w, 0), (x, 0), (x, 1), (w, 1), (x, 2), (w, 2),
                             (x, 3), (x, 4)]
                    for lst, i in order:
                        if lst is w:
                            lo, hi = W_BOUNDS[i]
                            nc.sync.dma_start(
                                w[i][:],
                                wd[lo * P:hi * P, :].rearrange(
                                    "(a p) j -> p a j", p=P
                                ),
                            )
                        else:
                            lo, hi = X_BOUNDS[i]
                            nc.sync.dma_start(
                                x[i][:],
                                xd[lo * P:hi * P, :].rearrange(
                                    "(a p) n -> p a n", p=P
                                ),
                            )
                    return w, x

                def _chunk(bounds, dc):
                    for i, (lo, hi) in enumerate(bounds):
                        if lo <= dc < hi:
                            return i, dc - lo
                    raise ValueError(dc)

                def wslice(w, dc, cols):
                    i, off = _chunk(W_BOUNDS, dc)
                    return w[i][:, off, cols]

                def xslice(x, dc):
                    i, off = _chunk(X_BOUNDS, dc)
                    return x[i][:, off, :]

                qpt = pacts.tile([P, NJ, N], BF16, name="qpt", tag="qpt")
                kpt = pacts.tile([P, NJ, N], BF16, name="kpt", tag="kpt")
                vext = pacts.tile([P, NKT, NH, HD + 1], BF16, name="vext",
                                  tag="vext")
                ot = pacts.tile([P, NJ, N], BF16, name="ot", tag="ot")

                # PSUM -> SBUF evacuation, alternating scalar/vector engines.
                def evac(idx, dst, src):
                    if idx % 2 == 0:
                        nc.scalar.activation(dst, src, AF.Copy)
                    else:
                        nc.vector.tensor_copy(dst, src)

                # --- Q/K projections, contraction-chunk outer over 8
                # concurrently accumulating PSUM banks (4 two-bank tiles).
                def qk_proj(nm, wd, xd, bias_t, dst):
                    w, x = load_wx(wd, xd, nm)
                    for cs in (range(3), range(3, NJ)):
                        groups = {
                            c: psum.tile([P, NQC, QW], F32, name="psp",
                                         tag="ps")
                            for c in cs
                        }
                        if use_bias:
                            for c in cs:
                                for qc in range(NQC):
                                    nc.tensor.matmul(
                                        groups[c][:, qc, :],
                                        bias_t[0:1, c * P:(c + 1) * P],
                                        ones_row[:], start=True, stop=False,
                                        skip_group_check=True,
                                    )
                        for dc in range(ND):
                            for c in cs:
                                for qc in range(NQC):
                                    nc.tensor.matmul(
                                        groups[c][:, qc, :],
                                        wslice(w, dc, slice(c * P, (c + 1) * P)),
                                        xslice(x, dc)[:, qc * QW:(qc + 1) * QW],
                                        start=(dc == 0 and not use_bias),
                                        stop=(dc == ND - 1),
                                        skip_group_check=True,
                                    )
                        for c in cs:
                            evac(
                                c, dst[:, c, :],
                                groups[c][:].rearrange("p a q -> p (a q)"),
                            )

                qk_proj("q", wq_d, qt_d, bqs_t if use_bias else None, qpt)
                qk_proj("k", wk_d, kt_d, bks_t if use_bias else None, kpt)

                # --- Attention blocks.  Head pairs are row-tiled in the PE
                # (even head partitions 0-63, odd 64-127); exps run over
                # two score banks at once ([128, 1024] fp32 -> bf16).
                def s_block(pr, qc, pts):
                    for h in (2 * pr, 2 * pr + 1):
                        pts[(h, qc)] = [
                            ppt.tile([P, 2, QW], BF16, name="pt", tag="pt")
                            for _ in range(NKT // 2)
                        ]
                    for kcp in range(NKT // 2):
                        pss = {}
                        for h in (2 * pr, 2 * pr + 1):
                            pss[h] = psum.tile([P, 2, QW], F32, name="pss",
                                               tag="ps")
                        for i in range(2):
                            kc = 2 * kcp + i
                            for h in (2 * pr, 2 * pr + 1):
                                off = HD * (h & 1)
                                nc.tensor.matmul(
                                    pss[h][:, i, :],
                                    kpt[off:off + HD, pr, kc * P:(kc + 1) * P],
                                    qpt[off:off + HD, pr, qc * QW:(qc + 1) * QW],
                                    start=True, stop=True,
                                    skip_group_check=True,
                                )
                        for h in (2 * pr, 2 * pr + 1):
                            if use_mask:
                                nc.vector.tensor_add(
                                    pss[h][:],
                                    pss[h][:],
                                    mb_t[:, 2 * kcp:2 * kcp + 2,
                                         qc * QW:(qc + 1) * QW],
                                )
                            nc.scalar.activation(
                                pts[(h, qc)][kcp][:], pss[h][:], AF.Exp,
                            )

                def pv_block(h, qc, pt):
                    po = psum.tile([HD + 1, QW], F32, name="po", tag="po",
                                   bufs=2)
                    for kc in range(NKT):
                        nc.tensor.matmul(
                            po[:], vext[:, kc, h, :],
                            pt[kc // 2][:, kc % 2, :],
                            start=(kc == 0), stop=(kc == NKT - 1),
                        )
                    rc = psml.tile([1, QW], F32, name="rc", tag="rc", bufs=2)
                    nc.vector.reciprocal(rc[:], po[HD:HD + 1, :])
                    rb = psml.tile([HD, QW], F32, name="rb", tag="rb", bufs=2)
                    nc.gpsimd.partition_broadcast(rb[:], rc[:])
                    off = HD * (h & 1)
                    nc.vector.tensor_mul(
                        ot[off:off + HD, h // 2, qc * QW:(qc + 1) * QW],
                        po[0:HD, :], rb[:],
                    )

                def fused_block(s_task, pv_task, pts):
                    """One drain cycle: score block for s_task with the
                    P@V pair for pv_task interleaved at kc-pair steps, so
                    the PE has ready P@V work whenever score matmuls wait
                    on an exp slot release."""
                    pr, qc = s_task
                    prv, qcv = pv_task
                    pta = pts.pop((2 * prv, qcv))
                    ptb = pts.pop((2 * prv + 1, qcv))
                    poa = psum.tile([HD + 1, QW], F32, name="po", tag="po",
                                    bufs=2)
                    pob = psum.tile([HD + 1, QW], F32, name="po", tag="po",
                                    bufs=2)
                    for h in (2 * pr, 2 * pr + 1):
                        pts[(h, qc)] = [
                            ppt.tile([P, 2, QW], BF16, name="pt", tag="pt")
                            for _ in range(NKT // 2)
                        ]
                    for kcp in range(NKT // 2):
                        pss = {}
                        for h in (2 * pr, 2 * pr + 1):
                            pss[h] = psum.tile([P, 2, QW], F32, name="pss",
                                               tag="ps")
                        for i in range(2):
                            kc = 2 * kcp + i
                            for h in (2 * pr, 2 * pr + 1):
                                off = HD * (h & 1)
                                nc.tensor.matmul(
                                    pss[h][:, i, :],
                                    kpt[off:off + HD, pr, kc * P:(kc + 1) * P],
                                    qpt[off:off + HD, pr, qc * QW:(qc + 1) * QW],
                                    start=True, stop=True,
                                    skip_group_check=True,
                                )
                        for h in (2 * pr, 2 * pr + 1):
                            if use_mask:
                                nc.vector.tensor_add(
                                    pss[h][:],
                                    pss[h][:],
                                    mb_t[:, 2 * kcp:2 * kcp + 2,
                                         qc * QW:(qc + 1) * QW],
                                )
                            nc.scalar.activation(
                                pts[(h, qc)][kcp][:], pss[h][:], AF.Exp,
                            )
                        for po_, pt_, hv in ((poa, pta, 2 * prv),
                                             (pob, ptb, 2 * prv + 1)):
                            for i in range(2):
                                kc = 2 * kcp + i
                                nc.tensor.matmul(
                                    po_[:], vext[:, kc, hv, :],
                                    pt_[kc // 2][:, kc % 2, :],
                                    start=(kc == 0), stop=(kc == NKT - 1),
                                )
                    for po_, hv in ((poa, 2 * prv), (pob, 2 * prv + 1)):
                        rc = psml.tile([1, QW], F32, name="rc", tag="rc",
                                       bufs=2)
                        nc.vector.reciprocal(rc[:], po_[HD:HD + 1, :])
                        rb = psml.tile([HD, QW], F32, name="rb", tag="rb",
                                       bufs=2)
                        nc.gpsimd.partition_broadcast(rb[:], rc[:])
                        off = HD * (hv & 1)
                        nc.vector.tensor_mul(
                            ot[off:off + HD, hv // 2, qcv * QW:(qcv + 1) * QW],
                            po_[0:HD, :], rb[:],
                        )

                pts = {}
                squeue = deque(
                    (pr, qc) for qc in range(NQC) for pr in range(NH // 2)
                )
                pvqueue = deque()

                def emit_s_one():
                    if squeue:
                        pr, qc = squeue.popleft()
                        s_block(pr, qc, pts)
                        pvqueue.append((pr, qc))

                # --- V projection (into [k, j] + ones column) in two
                # passes over 3 two-bank PSUM tiles, one score block
                # interleaved so ACT gets exp work during the vT DMA.
                vw, vx = load_wx(wv_d, vt_d, "v")
                nc.vector.memset(vext[:, :, :, HD:HD + 1], 1.0)
                for kcs in (range(0, 6), range(6, NKT)):
                    vgroups = {
                        kc: psum.tile([P, 2, QW], F32, name="psv", tag="ps")
                        for kc in kcs[::2]
                    }
                    if use_bias:
                        for kc in kcs:
                            nc.tensor.matmul(
                                vgroups[kc - kc % 2][:, kc % 2, :],
                                ones_row[0:1, 0:P], bvs_t[:],
                                start=True, stop=False,
                                skip_group_check=True,
                            )
                    for dc in range(ND):
                        for kc in kcs:
                            nc.tensor.matmul(
                                vgroups[kc - kc % 2][:, kc % 2, :],
                                xslice(vx, dc)[:, kc * P:(kc + 1) * P],
                                wslice(vw, dc, slice(0, J)),
                                start=(dc == 0 and not use_bias),
                                stop=(dc == ND - 1),
                                skip_group_check=True,
                            )
                        if kcs.start == 0 and dc == 3:
                            emit_s_one()
                    for i, kc in enumerate(kcs[::2]):
                        evac(
                            i, vext[:, kc:kc + 2, :, 0:HD],
                            vgroups[kc][:].rearrange(
                                "p a (h d) -> p a h d", h=NH
                            ),
                        )

                wo_t = pwo.tile([P, NJ, OUT], BF16, name="wo_t", tag="wo")
                nc.sync.dma_start(
                    wo_t[:], wo_d[:].rearrange("(a p) n -> p a n", p=P)
                )

                # --- Output projection (partial over this core's heads),
                # emitted per query half as soon as that half's O^T is done.
                # `split` chops the PSUM evacuation + store DMA into halves
                # (shorter serial tail); `alt` puts the copy on the scalar
                # engine (idle once the exps are done).
                def emit_final_qm(qm, split=False, alt=False):
                    ps = psum.tile([P, NQC, QW], F32, name="psf", tag="ps")
                    for oc in range(NQC):
                        for jc in range(NJ):
                            nc.tensor.matmul(
                                ps[:, oc, :],
                                ot[:, jc, qm * P:(qm + 1) * P],
                                wo_t[:, jc, oc * QW:(oc + 1) * QW],
                                start=(jc == 0), stop=(jc == NJ - 1),
                                skip_group_check=True,
                            )
                    rows = out_d[qm * P:(qm + 1) * P, :]
                    if split:
                        for oc in range(NQC):
                            oc_t = pout.tile([P, QW], F32, name="oct",
                                             tag="outh", bufs=2)
                            evac(oc if alt else oc + 1, oc_t[:], ps[:, oc, :])
                            nc.sync.dma_start(
                                rows[:, oc * QW:(oc + 1) * QW], oc_t[:]
                            )
                    else:
                        oc_t = pout.tile([P, OUT], F32, name="oct", tag="outc")
                        evac(0 if alt else 1, oc_t[:],
                             ps[:].rearrange("p a q -> p (a q)"))
                        nc.sync.dma_start(rows, oc_t[:])

                # --- drain remaining score blocks and P@V, software
                # pipelined (PV for block i emitted after block i+1); the
                # qc=0 half of the output projection interleaves with the
                # qc=1 score blocks.
                done_pv = 0
                finq = deque()
                prev = pvqueue.popleft()  # block emitted during V-proj
                while squeue:
                    cur = squeue.popleft()
                    fused_block(cur, prev, pts)
                    done_pv += 1
                    if done_pv == NH // 2:
                        finq.extend(range(4))  # qc0 output rows ready
                    if finq:
                        emit_final_qm(finq.popleft())
                    prev = cur
                while finq:
                    emit_final_qm(finq.popleft())
                pv_block(2 * prev[0], prev[1], pts.pop((2 * prev[0], prev[1])))
                pv_block(2 * prev[0] + 1, prev[1],
                         pts.pop((2 * prev[0] + 1, prev[1])))
                emit_final_qm(4, alt=True)
                emit_final_qm(5, alt=True)
                emit_final_qm(6, split=True, alt=True)
                emit_final_qm(7, split=True, alt=True)

            if reps == 1:
                _emit_rep()
            else:
                with tc.For_i(0, reps, 1):
                    _emit_rep()

    nc.compile()
    return nc


_NC_CACHE = {}


def _get_nc(use_mask: bool, use_bias: bool = False, reps: int = 1):
    key = (use_mask, use_bias, reps)
    if key not in _NC_CACHE:
        _NC_CACHE[key] = _build(use_mask, use_bias, reps)
    return _NC_CACHE[key]


def _group_weights(Wq, bq, Wk, bk, Wv, bv, Wo, g):
    """Per-head-group weight slices in per-core layout j = head*64 + d.

    The module splits heads as reshape(b, n, head_dim, NUM_HEADS), so
    global column d*NUM_HEADS + h belongs to (head h, dim d).
    """
    scale = float(NUM_HEADS * HD) ** -0.5
    cols = np.array(
        [d * NUM_HEADS + (NH * g + hl) for hl in range(NH) for d in range(HD)]
    )
    return {
        "wq": np.ascontiguousarray((Wq[:, cols] * scale).astype(NPBF)),
        "bqs": np.ascontiguousarray((bq[cols] * scale)[None, :].astype(NPBF)),
        "wk": np.ascontiguousarray(Wk[:, cols].astype(NPBF)),
        "bks": np.ascontiguousarray(bk[cols][None, :].astype(NPBF)),
        "wv": np.ascontiguousarray(Wv[:, cols].astype(NPBF)),
        "bvs": np.ascontiguousarray(bv[cols][None, :].astype(NPBF)),
        "wo": np.ascontiguousarray(Wo[cols, :].astype(NPBF)),
    }


def make_in_maps(q, k, v, attn_mask, Wq, bq, Wk, bk, Wv, bv, Wo, bo):
    """Shard the full inputs into 8 per-core input maps."""
    use_mask = not bool(np.all(np.asarray(attn_mask) == 1.0))
    use_bias = bool(
        np.any(np.asarray(bq)) or np.any(np.asarray(bk)) or np.any(np.asarray(bv))
    )
    gw = [_group_weights(Wq, bq, Wk, bk, Wv, bv, Wo, g) for g in range(2)]
    xt = [
        {
            "qt": np.ascontiguousarray(np.asarray(q[b]).T.astype(NPBF)),
            "kt": np.ascontiguousarray(np.asarray(k[b]).T.astype(NPBF)),
            "vt": np.ascontiguousarray(np.asarray(v[b]).T.astype(NPBF)),
        }
        for b in range(B)
    ]
    mb = None
    if use_mask:
        mb = np.ascontiguousarray(
            (-100000000.0 * (1.0 - np.asarray(attn_mask))).T, dtype=np.float32
        )
    in_maps = []
    for c in range(8):
        b, g = divmod(c, 2)
        m = dict(xt[b])
        m.update(gw[g])
        if not use_bias:
            for nm in ("bqs", "bks", "bvs"):
                m.pop(nm, None)
        if use_mask:
            m["mb"] = mb
        in_maps.append(m)
    return in_maps, use_mask, use_bias


def kernel(q, k, v, attn_mask, Wq, bq, Wk, bk, Wv, bv, Wo, bo):
    in_maps, use_mask, use_bias = make_in_maps(
        q, k, v, attn_mask, Wq, bq, Wk, bk, Wv, bv, Wo, bo
    )
    nc = _get_nc(use_mask, use_bias)
    res = run_bass_kernel_spmd(nc, in_maps, list(range(8)))
    out = np.empty((B, N, OUT), np.float32)
    bo = np.asarray(bo, np.float32)
    for b in range(B):
        out[b] = res.results[2 * b]["out"] + res.results[2 * b + 1]["out"] + bo
    return out
